# revision 14
# baseline (speedup 1.0000x reference)
"""Mixture-of-Experts (E=8, top-2) — F-sliced Trainium2 Bass kernel.

Strategy (intermediate-dim sharding; perfectly load-balanced):
  * Host computes the router (logits -> top-2 -> softmax) in numpy and sorts
    the 2*T (token, slot) pairs by expert.
  * Core c keeps ALL 8 experts' weights resident, but only the F-column slice
    [512c, 512(c+1)) of each — 16.8 MB of bf16, fits SBUF.  Every core streams
    ALL pairs through its slice:  y_part = w2[e][:, fs].T' @ gelu(w1[e][fs] @ x
    + b1[fs]).  Partials are evicted in bf16 and summed on the host (+ b2 and
    the top-2 prob combine).
  * Because every core runs every pair, the work is identical on all cores no
    matter how tokens route: 16384 matmul columns each, zero capacity padding.
    The chunk schedule (chunks never straddle an expert boundary) is baked
    into the program from the exact per-expert counts.

Pipeline notes (from perfetto analysis of the 470us baseline):
  * exec window = [first const-init MEMSET ~5.8us, last teardown instr]; the
    ~10us framework teardown (250 per-engine sem clears) is fixed cost.
  * Ramp: the first matmul needs only w1[e0] m-tile 0 + x chunk 0.  w1 is
    packed m-major so m-tile 0 is one contiguous 2KB/partition run; it rides
    sync first while b1+x0+x1 ride scalar and w2[e0] rides gpsimd — three
    queues in parallel get the first matmul going ~9.5us (was 15.1us).
  * Each chunk's first MLP2 group stalls ~0.2us on the last MLP1 activation;
    the next chunk's MLP1 m0 group is emitted in between to hide it.

DMA layout note: the HW DGE queues are packet-rate limited (~110 packets/us),
so every DRAM-side access pattern here is packed host-side to be ONE
contiguous run per partition (128 packets per transfer, ~8 KB each) — flat
chunk-major x/y streams, partition-major weights.

Device layout (per core, SPMD — same program, per-core weight slices):
  xtf  [P, KD*TP]    bf16  chunk-major packed tokens (see _pack_x)
  w1s  [E, P, MF*KD*128] bf16  m-major w1[e].T column-slice (m-tile = one run)
  w2s  [E, P, KS*D]  bf16  partition-major w2[e].T row-slice
  b1s  [P, E*MF]     f32   b1 slice as per-partition bias table
  ytf  [P, MD*TP]    bf16  chunk-major packed partial outputs
"""

import numpy as np
from contextlib import ExitStack

from ml_dtypes import bfloat16

import concourse.bacc as bacc
import concourse.tile as tile
import concourse.mybir as mybir
from concourse.bass_utils import run_bass_kernel_spmd

P = 128
D = 1024
F = 4096
E = 8
TOPK = 2
B, S = 4, 2048
T = B * S
TP = TOPK * T      # 16384 (token, slot) pairs, each a matmul column

FS = F // E        # 512  F-slice width per core
NT = 512           # max tokens per chunk (matmul moving free dim limit)

KD = D // P        # 8  k-tiles for MLP1 (contract D)
MF = FS // P       # 4  m-tiles for MLP1 output (F slice)
KS = FS // P       # 4  k-tiles for MLP2 (contract F slice)
MD = D // P        # 8  m-tiles for MLP2 output (D)

_prog_cache: dict = {}
ACT_FUNC = None  # default: Gelu; sim_check overrides (CoreSim lacks Gelu)
last_results = None  # BassKernelResults of the most recent run (for test harness)
trace_kwargs: dict = {}  # test harness can set e.g. {"trace": True}


def _split_even(total, lead=()):
    """Split `total` cols into near-equal chunks <= NT (plus optional small
    leading chunks).  Equal widths keep every matmul >= ~128 cols so the PE's
    weight preload stays hidden; a 512*k + tiny-remainder split does not."""
    sizes = []
    for sz in lead:
        if total <= sz:
            break
        sizes.append(sz)
        total -= sz
    k = -(-total // NT)
    base, extra = divmod(total, k)
    sizes.extend([base + 1] * extra + [base] * (k - extra))
    return sizes


def _schedule(counts):
    """Chunk schedule [(expert, col_offset, width), ...] — no chunk straddles
    an expert boundary; widths <= NT; total width == sum(counts).

    The first chunks are 128/384 cols so the PE can start once ~0.5 MB of
    DMA lands (the early chunks run DMA-starved anyway, so their weaker
    weight-preload hiding is free); the last chunk is small so the
    MLP2+evict+writeback drain after the final MLP1 is short."""
    sched = []
    off = 0
    last_e = max((e for e in range(E) if counts[e] > 0), default=0)
    for e in range(E):
        if counts[e] == 0:
            continue
        sizes = _split_even(int(counts[e]), lead=(128, 384) if not sched else ())
        if e == last_e and sizes[-1] > 256:
            sizes = sizes[:-1] + [sizes[-1] - 128, 128]
        for n in sizes:
            sched.append((e, off, n))
            off += n
    return tuple(sched)


def _build_program(sched):
    """Build + compile the SPMD F-sliced all-experts program."""
    bf16 = mybir.dt.bfloat16
    f32 = mybir.dt.float32

    nc = bacc.Bacc(
        "TRN2",
        target_bir_lowering=False,
        debug=False,
        enable_asserts=False,
        num_devices=E,
    )

    xtf = nc.dram_tensor("xtf", [P, KD * TP], bf16, kind="ExternalInput").ap()
    w1s = nc.dram_tensor("w1s", [E, P, MF * KD * P], bf16, kind="ExternalInput").ap()
    w2s = nc.dram_tensor("w2s", [E, P, KS * D], bf16, kind="ExternalInput").ap()
    b1s = nc.dram_tensor("b1s", [P, E * MF], f32, kind="ExternalInput").ap()
    ytf = nc.dram_tensor("ytf", [P, MD * TP], bf16, kind="ExternalOutput").ap()

    with tile.TileContext(nc) as tc, ExitStack() as ctx:
        wpool = ctx.enter_context(tc.tile_pool(name="wpool", bufs=1))
        xpool = ctx.enter_context(tc.tile_pool(name="xpool", bufs=3))
        hpool = ctx.enter_context(tc.tile_pool(name="hpool", bufs=3))
        ypool = ctx.enter_context(tc.tile_pool(name="ypool", bufs=2))
        ps1 = ctx.enter_context(tc.tile_pool(name="ps1", bufs=3, space="PSUM"))
        ps2 = ctx.enter_context(tc.tile_pool(name="ps2", bufs=3, space="PSUM"))

        b1_sb = wpool.tile([P, E, MF], f32, name="b1sb")
        w1_sb = [wpool.tile([P, MF, KD * P], bf16, name=f"w1_{e}") for e in range(E)]
        w2_sb = [wpool.tile([P, KS, D], bf16, name=f"w2_{e}") for e in range(E)]

        def load_w(e, eng):
            eng.dma_start(
                out=w1_sb[e][:, :, :],
                in_=w1s[e].rearrange("p (m x) -> p m x", m=MF),
            )
            eng.dma_start(
                out=w2_sb[e][:, :, :],
                in_=w2s[e].rearrange("p (k d) -> p k d", k=KS),
            )

        # Ramp: the ~430 GB/s per-core HBM pipe is SHARED across all in-flight
        # DMA transfers (a single transfer tops out ~200 GB/s), so the ramp is
        # bandwidth-bound — the byte stream must arrive in consumption order,
        # ~2 transfers in flight.  Two FIFO lanes:
        #   sync:   w1[e0] m0..m3 (256KB each, contiguous in the m-major
        #           layout), then the x stream x2, x3, ...
        #   scalar: x0, b1, x1, then w2[e0], then the y writebacks
        # The tile scheduler reorders same-engine DMAs it deems independent,
        # so x1 is chained behind x0 and w2[e0] behind x1 with dummy gpsimd
        # copies (a real RAW+WAW edge pins the order).  With the depth-2
        # MLP1 lookahead below, w2[e0] isn't needed until ~20us.
        e_first = sched[0][0]
        w1_first_dram = w1s[e_first].rearrange("p (m x) -> p m x", m=MF)
        # first half (m0+m1) as one 4KB/partition transfer: bigger packets
        # win a larger share of the shared HBM pipe during the ramp
        nc.sync.dma_start(
            out=w1_sb[e_first][:, 0 : MF // 2, :],
            in_=w1_first_dram[:, 0 : MF // 2, :],
        )
        nc.scalar.dma_start(out=b1_sb[:, :, :], in_=b1s.rearrange("p (e m) -> p e m", e=E))

        def mlp1_group(e, m, n, x_sb, h_sb):
            pt = ps1.tile([P, NT], f32, name="p1")
            for k in range(KD):
                nc.tensor.matmul(
                    pt[:, :n],
                    lhsT=w1_sb[e][:, m, k * P : (k + 1) * P],
                    rhs=x_sb[:, k * n : (k + 1) * n],
                    start=(k == 0),
                    stop=(k == KD - 1),
                )
            nc.scalar.activation(
                h_sb[:, m, :n],
                pt[:, :n],
                ACT_FUNC or mybir.ActivationFunctionType.Gelu,
                bias=b1_sb[:, e, m : m + 1],
            )

        def mlp2_all(e, off, n, h_sb):
            # MLP2 partial: y[D, n] = w2s[e].T @ h, bf16 out (b2 on host),
            # written back in halves so the drain after the last MLP1 is short
            y_sb = ypool.tile([P, MD * NT], bf16, name="ytile")
            for m in range(MD):
                pt = ps2.tile([P, NT], f32, name="p2")
                for k in range(KS):
                    nc.tensor.matmul(
                        pt[:, :n],
                        lhsT=w2_sb[e][:, k, m * P : (m + 1) * P],
                        rhs=h_sb[:, k, :n],
                        start=(k == 0),
                        stop=(k == KS - 1),
                    )
                nc.vector.tensor_copy(
                    out=y_sb[:, m * n : (m + 1) * n], in_=pt[:, :n]
                )
                if m == MD // 2 - 1 or m == MD - 1:
                    h0 = (m + 1 - MD // 2) * n
                    nc.scalar.dma_start(
                        out=ytf[:, MD * off + h0 : MD * off + (m + 1) * n],
                        in_=y_sb[:, h0 : (m + 1) * n],
                    )

        def x_load(ci, eng):
            e, off, n = sched[ci]
            x_sb = xpool.tile([P, KD * NT], bf16, name="xtile")
            eng.dma_start(
                out=x_sb[:, : KD * n], in_=xtf[:, KD * off : KD * (off + n)]
            )
            return x_sb

        # Software pipeline, MLP1 running two chunks ahead of MLP2: the PE
        # order is  ... m1..m3(c), m0(c+1), MLP2(c-1) ...   The depth-2
        # lookahead (a) hides the act(c, m3) -> MLP2(c) semaphore latency
        # behind a full chunk of MLP1, and (b) defers the first MLP2 (and so
        # the w2[e0] bytes) to ~22us, letting the ramp DMA prioritize
        # w1[e0] + x0..x2.
        x_cur = x_load(0, nc.sync)  # chunk 0's x rides sync right after w1m0m1
        h_cur = hpool.tile([P, KS, NT], bf16, name="htile")
        mlp1_group(sched[0][0], 0, sched[0][2], x_cur, h_cur)
        # second half (m2+m3) lands while m0/m1 compute
        nc.sync.dma_start(
            out=w1_sb[e_first][:, MF // 2 :, :],
            in_=w1_first_dram[:, MF // 2 :, :],
        )
        for m in range(1, MF):
            mlp1_group(sched[0][0], m, sched[0][2], x_cur, h_cur)

        # x1 rides scalar (sync is still draining w1[e0] m-tiles).  The bulk
        # weights ride gpsimd, paced behind chunk 0's last MLP1 activation:
        # the dummy copies put a real RAW(h0) + WAW(w tile) dependency on
        # each DMA (engine program order alone gets rescheduled), keeping
        # HBM free for the critical x/w1[e0] window.
        x_nxt = x_load(1, nc.scalar)
        for ep in range(E):
            if ep == e_first:
                continue
            nc.gpsimd.tensor_copy(
                out=w1_sb[ep][:, 0, 0:1], in_=h_cur[:, MF - 1, 0:1]
            )
            nc.gpsimd.tensor_copy(
                out=w2_sb[ep][:, 0, 0:1], in_=h_cur[:, MF - 1, 0:1]
            )
            load_w(ep, nc.gpsimd)
        h_nxt = hpool.tile([P, KS, NT], bf16, name="htile")
        mlp1_group(sched[1][0], 0, sched[1][2], x_nxt, h_nxt)
        # w2[e0] rides scalar chained behind x1 (the copy into its tile makes
        # the DMA WAW-dependent on x1's arrival) so the tile scheduler cannot
        # hoist the 1MB transfer ahead of x1 — it isn't needed until MLP2(c0)
        # at ~20us.
        nc.gpsimd.tensor_copy(out=w2_sb[e_first][:, 0, 0:1], in_=x_nxt[:, 0:1])
        nc.scalar.dma_start(
            out=w2_sb[e_first][:, :, :],
            in_=w2s[e_first].rearrange("p (k d) -> p k d", k=KS),
        )

        x_prv, h_prv = x_cur, h_cur
        x_cur, h_cur = x_nxt, h_nxt
        for ci in range(1, len(sched)):
            e, off, n = sched[ci]
            for m in range(1, MF):
                mlp1_group(e, m, n, x_cur, h_cur)
            if ci + 1 < len(sched):
                x_nxt = x_load(ci + 1, nc.sync)
                h_nxt = hpool.tile([P, KS, NT], bf16, name="htile")
                mlp1_group(sched[ci + 1][0], 0, sched[ci + 1][2], x_nxt, h_nxt)
            ep_, op_, np_ = sched[ci - 1]
            mlp2_all(ep_, op_, np_, h_prv)
            x_prv, h_prv = x_cur, h_cur
            x_cur, h_cur = x_nxt, h_nxt
        e, off, n = sched[-1]
        mlp2_all(e, off, n, h_prv)

    nc.compile()
    return nc


def _get_program(sched):
    if sched not in _prog_cache:
        _prog_cache[sched] = _build_program(sched)
    return _prog_cache[sched]


def _route(xf: np.ndarray, router_w: np.ndarray):
    """Top-2 routing identical to the reference (ties -> lower expert idx).

    Logits in fp64 so the selection is independent of BLAS blocking/threads
    (top-2 gaps in this regime are >= ~3e-6; fp64 noise is ~1e-15).
    """
    logits = xf.astype(np.float64) @ router_w.T.astype(np.float64)  # [T, E]
    idx = np.argsort(-logits, axis=1, kind="stable")[:, :TOPK]
    vals = np.take_along_axis(logits, idx, axis=1)
    vals = vals - vals.max(axis=1, keepdims=True)
    ev = np.exp(vals)
    probs = (ev / ev.sum(axis=1, keepdims=True)).astype(np.float32)
    return idx.astype(np.int64), probs


def _pack_x(xf_bf, order, sched):
    """Pack gathered tokens chunk-major: chunk (off, n) occupies xtf columns
    [KD*off, KD*(off+n)), laid out [KD, n] so each partition's slice is one
    contiguous 2*KD*n-byte run."""
    xtf = np.empty((P, KD * TP), dtype=bfloat16)
    gathered = xf_bf[order // 2]                      # [TP, D]
    for _, off, n in sched:
        blk = gathered[off : off + n].T               # [D, n]
        blk = blk.reshape(KD, P, n).transpose(1, 0, 2).reshape(P, KD * n)
        xtf[:, KD * off : KD * (off + n)] = blk
    return xtf


def _unpack_y(acc, sched):
    """Inverse of the ytf packing: returns accT [TP, D] (pair-major)."""
    accT = np.empty((TP, D), dtype=np.float32)
    for _, off, n in sched:
        blk = acc[:, MD * off : MD * (off + n)].reshape(P, MD, n)
        accT[off : off + n] = blk.transpose(1, 0, 2).reshape(D, n).T
    return accT


def kernel(x, router_w, w1, b1, w2, b2):
    global last_results

    x = np.asarray(x, dtype=np.float32)
    router_w = np.asarray(router_w, dtype=np.float32)
    w1 = np.asarray(w1, dtype=np.float32)
    b1 = np.asarray(b1, dtype=np.float32)
    w2 = np.asarray(w2, dtype=np.float32)
    b2 = np.asarray(b2, dtype=np.float32)

    orig_shape = x.shape
    xf = x.reshape(-1, D)

    idx, probs = _route(xf, router_w)

    # Group the (token, k) pairs by expert; gpos = column in the sorted order.
    flat_e = idx.ravel()  # entry j corresponds to token j//2, slot j%2
    order = np.argsort(flat_e, kind="stable")
    counts = np.bincount(flat_e, minlength=E)
    starts = np.zeros(E + 1, dtype=np.int64)
    np.cumsum(counts, out=starts[1:])
    rank = np.empty(TP, dtype=np.int64)
    rank[order] = np.arange(TP, dtype=np.int64) - starts[flat_e[order]]
    gpos = (rank + starts[flat_e]).reshape(T, TOPK)

    sched = _schedule(counts)
    nc = _get_program(sched)

    xtf = _pack_x(xf.astype(bfloat16), order, sched)
    in_maps = []
    for c in range(E):
        fs = slice(c * FS, (c + 1) * FS)
        # m-major: [E, P, MF*KD*128] with w1c[e, p, (m, k, j)] =
        # w1[e].T[k*128+p, m*128+j] — m-tile m is one contiguous run.
        w1c = np.ascontiguousarray(
            w1[:, fs, :].transpose(0, 2, 1)        # [E, D, FS]
            .reshape(E, KD, P, MF, P).transpose(0, 2, 3, 1, 4)
            .reshape(E, P, MF * KD * P)
        ).astype(bfloat16)
        w2c = np.ascontiguousarray(
            w2[:, :, fs].transpose(0, 2, 1)        # [E, FS, D]
            .reshape(E, KS, P, D).transpose(0, 2, 1, 3)
            .reshape(E, P, KS * D)
        ).astype(bfloat16)
        b1c = np.ascontiguousarray(
            b1[:, fs].reshape(E, MF, P).transpose(2, 0, 1).reshape(P, E * MF)
        )
        in_maps.append({"xtf": xtf, "w1s": w1c, "w2s": w2c, "b1s": b1c})

    res = run_bass_kernel_spmd(nc, in_maps, core_ids=list(range(E)), **trace_kwargs)
    last_results = res

    acc = np.zeros((P, MD * TP), dtype=np.float32)
    for r in res.results:
        acc += np.asarray(r["ytf"]).astype(np.float32)
    accT = _unpack_y(acc, sched)                                  # [TP, D]
    out = probs[:, 0:1] * (accT[gpos[:, 0]] + b2[idx[:, 0]])
    out += probs[:, 1:2] * (accT[gpos[:, 1]] + b2[idx[:, 1]])
    return out.astype(np.float32).reshape(orig_shape)
